# revision 71
# baseline (speedup 1.0000x reference)
"""Trainium2 Bass kernel for nn_CrossAtt (cross-attention + concat + residual +
3x3 conv + BN + ReLU), data-parallel over (batch, row-group) across 8 cores.

Sharding: core i -> batch b = i//4, row-group rg = i%4 (16 output rows each).
Each core computes both attention branches for an 18-row extended window
(16 rows + 1 halo row each side, zero-masked at image edges), the fused
residual/concat outputs, and the 3x3 conv + BN + ReLU on its 16 rows.

v3 design (cost-model driven, Act-engine pacing):
- all matmul operands bf16; exp tiles + vT fp8e4 -> AV/den matmuls DoubleRow
- 4 passes: A=(br0,h1) overlapped with projection streaming, (br0,h0),
  (br1,h0), (br1,h1)+conv section 0 interleaved; conv section 1 in tail
- PIPELINED emission: S(p+1) emitted before AVden(p) so the in-order PE
  queue never blocks exp(p+1) behind AVden(p)'s wait on exp(p)
- pass-boundary interleaving: next pass's first 2 pairs of S+exp emitted
  before the previous pass's normalize; PSUM pools swapped exactly between
- normalize: gamma folded into the reciprocal broadcast, bv folded into the
  vT copy (softmax weights sum to 1), residual concat-half emitted early
- head: DMA priority order (wqk, x1e, xt chunks, x2e, consts late, wcat
  split into 4 mid-stream chunks), PE warm-up matmuls during DMA wait
- PSUM: head 8 = psk2+psv2+sstA2+avA1+denA1; h0 8 = sst4+av2+den2;
  pass4 8 = sst4+av1+den1+psy2
"""

import sys

sys.path.insert(0, "/opt/trn_rl_repo")

import numpy as np
import ml_dtypes

import concourse.bacc as bacc
import concourse.tile as tile
from concourse import mybir
from concourse.bass_utils import run_bass_kernel_spmd

F32 = mybir.dt.float32
BF16 = mybir.dt.bfloat16
FP8 = mybir.dt.float8e4
AF = mybir.ActivationFunctionType
ALU = mybir.AluOpType
DR = mybir.MatmulPerfMode.DoubleRow

B, C, H, W = 2, 256, 64, 64
NW = H * W  # 4096 key/value positions
RE = 18  # extended rows per core
NE = RE * W  # 1152 query positions per core
D_QK, D_V = 16, 128
N_CORES = 8
BN_EPS = 1e-5
HLEN = (640, 512)  # window halves: rows 0..9 | rows 10..17
HOFF = (0, 640)
EXP_BIAS = -3.5  # exp(S/4 - 3.5): keeps fp8 weights < 240; cancels in softmax

_PROG_CACHE: dict = {}


def _jchunks(hlen):
    # bank-aligned output chunks (PSUM bank = 512 f32)
    return [(0, 512), (512, hlen - 512)] if hlen > 512 else [(0, hlen)]


def _build_program(gamma: float):
    nc = bacc.Bacc("TRN2", target_bir_lowering=False, debug=False, num_devices=N_CORES)

    def din(name, shape, dt=BF16):
        return nc.dram_tensor(name, shape, dt, kind="ExternalInput").ap()

    def dout(name, shape):
        return nc.dram_tensor(name, shape, F32, kind="ExternalOutput").ap()

    x1f = din("x1f", [C, NW])
    x2f = din("x2f", [C, NW])
    x1e = din("x1e", [C, NE])
    x2e = din("x2e", [C, NE])
    maskd = din("maskd", [128, NE])
    wqkd = din("wqkd", [128, 64])
    wvd = din("wvd", [128, 256])
    wcatd = din("wcatd", [128, 4608])
    # packed small params: [:,0:4]=bn scale/beta (lo|hi), [:,4]=bv,
    # [0:16,5]=bq, [0:16,6]=bk
    bmiscd = din("bmiscd", [128, 8], F32)
    o1 = dout("o1", [C, 1024])
    o2 = dout("o2", [C, 1024])
    feat = dout("feat", [C, 1024])

    xf = [x1f, x2f]
    xe_d = [x1e, x2e]
    od = [o1, o2]

    with tile.TileContext(nc) as tc:
        with (
            tc.tile_pool(name="constp", bufs=1) as constp,
            tc.tile_pool(name="projp", bufs=1) as projp,
            tc.tile_pool(name="outp", bufs=1) as outp,
            tc.tile_pool(name="etp", bufs=6) as etp,
            tc.tile_pool(name="natp", bufs=2) as natp,
        ):
            # ---- SBUF tiles ----
            wqk_sb = constp.tile([128, 64], BF16, name="wqk_sb")
            wv_sb = constp.tile([128, 256], BF16, name="wv_sb")
            bmisc_sb = constp.tile([128, 8], F32, name="bmisc_sb")
            mask_sb = constp.tile([128, NE], BF16, name="mask_sb")
            wcat_sb = constp.tile([128, 4608], BF16, name="wcat_sb")
            xe_sb = [
                constp.tile([128, 2 * NE], BF16, name=f"xe_sb{i}") for i in range(2)
            ]
            ebias = constp.tile([128, 1], F32, name="ebias")
            ones8 = constp.tile([128, 32], FP8, name="ones8")
            ones1b = constp.tile([1, 128], BF16, name="ones1b")

            k_r = [projp.tile([16, NW], BF16, name=f"k_r{i}") for i in range(2)]
            q_r = [projp.tile([16, NE], BF16, name=f"q_r{i}") for i in range(2)]
            vT = [projp.tile([128, NW], FP8, name=f"vT{i}") for i in range(2)]

            spad = []
            for cc in range(2):
                sp = outp.tile([128, RE, 66], BF16, name=f"spad{cc}")
                spad.append(sp)
            out_e = [
                outp.tile([128, 2 * NE], F32, name=f"out_e{br}") for br in range(2)
            ]

            # ---- head DMAs ----
            # HWDGE queues carry ONLY the critical path (x1e + xt stream +
            # stores): a dma_start holds its engine's SEQ until its HWDGE
            # slot completes (~1.3us), so everything else goes via SWDGE
            # (Pool engine), gated to avoid DMA_ENGINES contention.
            nc.sync.dma_start(wqk_sb[:], wqkd[:])
            # x1e h1-half first (one merged DMA over both channel planes):
            # pass A's q projection (cols 640:1152) gates the first S matmul
            xe_v = [
                t[:].rearrange("p (two n) -> p two n", two=2) for t in xe_sb
            ]
            xe_dv = [
                x.rearrange("(two p) n -> p two n", two=2) for x in xe_d
            ]
            nc.sync.dma_start(xe_v[0][:, :, 640:NE], xe_dv[0][:, :, 640:NE])

            # Pool/SWDGE queue: small consts now; x2e/wcat/mask deferred to
            # nt-loop hooks behind gate copies that write INTO the DMA's
            # destination tile (WAW dep - scheduler can't hoist the DMA)
            nc.gpsimd.dma_start(bmisc_sb[:], bmiscd[:])
            nc.gpsimd.dma_start(wv_sb[:], wvd[:])
            nc.gpsimd.memset(ebias[:], EXP_BIAS)
            nc.gpsimd.memset(ones8[:], 1.0)
            nc.gpsimd.memset(ones1b[:], 1.0)
            for cc in range(2):
                nc.gpsimd.memset(spad[cc][:], 0.0)

            onesv = ones8[:].rearrange("p (two m) -> p two m", two=2)

            # ---- PSUM pools (head phase) ----
            # two independent stacks: left = streaming/s_t ring, right = av/den
            cm_pskp = tc.tile_pool(name="pskp", bufs=2, space="PSUM", side="left")
            pskp = cm_pskp.__enter__()
            cm_psvp = tc.tile_pool(name="psvp", bufs=2, space="PSUM", side="left")
            psvp = cm_psvp.__enter__()
            cm_sstpA = tc.tile_pool(name="sstpA", bufs=2, space="PSUM", side="right")
            sstpA = cm_sstpA.__enter__()
            cm_avpA = tc.tile_pool(name="avpA", bufs=1, space="PSUM", side="right")
            avpA = cm_avpA.__enter__()
            cm_denpA = tc.tile_pool(name="denpA", bufs=1, space="PSUM", side="right")
            denpA = cm_denpA.__enter__()

            avA = [avpA.tile([128, 512], F32, name="avA0")]
            denA = [denpA.tile([16, 512], F32, name="denA0")]

            # tiny dummy Exp at t=0: pulls the ACT_TABLE_LOAD off the first
            # real exp's critical path
            dummy = constp.tile([1, 1], F32, name="dummy")
            nc.scalar.activation(dummy[:], ebias[0:1, 0:1], AF.Exp)

            # ---- emission helpers ----
            def emit_q_chunk(i, q0, q1, pool=None):
                # q_r[i] cols q0:q1 (<=512 wide, PSUM bank); post-stream
                # chunks ride the sstpA ring so pskp can close early
                psq = (pool or pskp).tile(
                    [16, q1 - q0], F32, name="psk",
                    tag="psk" if pool is None else "s_t",
                )
                for cc in range(2):
                    nc.tensor.matmul(
                        psq[:],
                        wqk_sb[:, 16 * cc : 16 * cc + 16],
                        xe_sb[i][:, NE * cc + q0 : NE * cc + q1],
                        start=(cc == 0),
                        stop=(cc == 1),
                    )
                nc.vector.tensor_scalar_add(
                    q_r[i][:, q0:q1], psq[:], bmisc_sb[0:16, 5:6]
                )

            def emit_residual(br, h):
                # concat upper half: out_e[:,NE:2NE] = gamma*x_lo + x_hi
                c0, c1 = (0, 640) if h == 0 else (640, NE)
                nc.vector.scalar_tensor_tensor(
                    out_e[br][:, NE + c0 : NE + c1],
                    xe_sb[br][:, c0:c1],
                    gamma,
                    xe_sb[br][:, NE + c0 : NE + c1],
                    ALU.mult,
                    ALU.add,
                )

            def emit_S_exp(br, h, sstp_, pair):
                hlen, hoff = HLEN[h], HOFF[h]
                jch = _jchunks(hlen)
                et_t = etp.tile([128, 2 * hlen], FP8, name="et")
                for par in range(2):
                    mi = 2 * pair + par
                    s_t = sstp_.tile([128, hlen], F32, name="s_t", tag="s_t")
                    for jo, jl in jch:
                        nc.tensor.matmul(
                            s_t[:, jo : jo + jl],
                            k_r[br][:, mi * 128 : mi * 128 + 128],
                            q_r[br][:, hoff + jo : hoff + jo + jl],
                            start=True,
                            stop=True,
                        )
                    nc.scalar.activation(
                        et_t[:, par * hlen : (par + 1) * hlen],
                        s_t[:],
                        AF.Exp,
                        bias=ebias[:],
                        scale=0.25,
                    )
                return et_t

            def emit_AVden(br, h, av, den, et_t, pair):
                hlen = HLEN[h]
                jch = _jchunks(hlen)
                etv = et_t[:].rearrange("p (two n) -> p two n", two=2)
                vv = vT[br][:, pair * 256 : (pair + 1) * 256].rearrange(
                    "p (two m) -> p two m", two=2
                )
                for j, (jo, jl) in enumerate(jch):
                    nc.tensor.matmul(
                        av[j][:],
                        vv,
                        etv[:, :, jo : jo + jl],
                        start=(pair == 0),
                        stop=(pair == 15),
                        perf_mode=DR,
                        skip_group_check=True,
                    )
                    nc.tensor.matmul(
                        den[j][:],
                        onesv[:, :, 0:16],
                        etv[:, :, jo : jo + jl],
                        start=(pair == 0),
                        stop=(pair == 15),
                        perf_mode=DR,
                        skip_group_check=True,
                    )

            def emit_normalize(br, h, av, den, bb_pool):
                hlen, hoff = HLEN[h], HOFF[h]
                jch = _jchunks(hlen)
                recip_f = natp.tile([1, hlen], F32, name="recip_f")
                for j, (jo, jl) in enumerate(jch):
                    nc.vector.reciprocal(recip_f[0:1, jo : jo + jl], den[j][0:1, :])
                recip_b = natp.tile([1, hlen], BF16, name="recip_b")
                nc.vector.tensor_scalar_mul(recip_b[:], recip_f[:], gamma)
                bb = bb_pool.tile([128, hlen], F32, name="bb", tag="s_t")
                for jo, jl in jch:
                    nc.tensor.matmul(
                        bb[:, jo : jo + jl],
                        ones1b[0:1, :],
                        recip_b[0:1, jo : jo + jl],
                        start=True,
                        stop=True,
                    )
                for j, (jo, jl) in enumerate(jch):
                    bcp = natp.tile([128, jl], F32, name="bcp")
                    nc.vector.tensor_copy(bcp[:], bb[:, jo : jo + jl])
                    attn_t = natp.tile([128, jl], F32, name="attn_t")
                    nc.vector.tensor_mul(attn_t[:], av[j][:], bcp[:])
                    c0 = hoff + jo
                    nc.vector.tensor_add(
                        out_e[br][:, c0 : c0 + jl],
                        attn_t[:],
                        xe_sb[br][:, c0 : c0 + jl],
                    )

            def emit_store(br, h):
                if h == 0:
                    nc.sync.dma_start(od[br][0:128, 0:576], out_e[br][:, 64:640])
                    nc.sync.dma_start(
                        od[br][128:256, 0:576], out_e[br][:, NE + 64 : NE + 640]
                    )
                else:
                    nc.sync.dma_start(od[br][0:128, 576:1024], out_e[br][:, 640:1088])
                    nc.sync.dma_start(
                        od[br][128:256, 576:1024], out_e[br][:, NE + 640 : NE + 1088]
                    )

            def emit_spad(sec, cc):
                # sec 0: rows 0..9 <- cols 0:640 ; sec 1: rows 10..17 <- cols 640:1152
                # cc=1 (upper channels) depends only on the early residuals
                r0, r1 = (0, 10) if sec == 0 else (10, 18)
                c0, c1 = (0, 640) if sec == 0 else (640, 1152)
                nr = r1 - r0
                sm = outp.tile([128, 640], BF16, name="sm", bufs=2)
                nc.vector.tensor_add(
                    sm[:, 0 : c1 - c0],
                    out_e[0][:, NE * cc + c0 : NE * cc + c1],
                    out_e[1][:, NE * cc + c0 : NE * cc + c1],
                )
                nc.vector.tensor_mul(
                    spad[cc][:, r0:r1, 1:65],
                    sm[:, 0 : c1 - c0].rearrange("p (r c) -> p r c", r=nr),
                    mask_sb[:, c0:c1].rearrange("p (r c) -> p r c", r=nr),
                )

            conv_jobs = [
                (oc, t, cc) for oc in range(2) for t in range(9) for cc in range(2)
            ]

            def emit_conv_mm2(hh, psy, oc, t, cc, start, stop):
                dy, dx = t // 3, t % 3
                nc.tensor.matmul(
                    psy[oc][:],
                    wcat_sb[
                        :,
                        2304 * cc + 256 * t + 128 * oc : 2304 * cc
                        + 256 * t
                        + 128 * oc
                        + 128,
                    ],
                    spad[cc][:, 8 * hh + dy : 8 * hh + dy + 8, dx : dx + 64],
                    start=start,
                    stop=stop,
                    skip_group_check=True,
                )

            def emit_conv_mm(hh, psy, k):
                oc, t, cc = conv_jobs[k]
                emit_conv_mm2(
                    hh, psy, oc, t, cc, start=(t == 0 and cc == 0),
                    stop=(t == 8 and cc == 1),
                )

            def emit_conv_out(hh, psy, oc, split=1):
                # BN+ReLU straight to f32 (no bf16 round-trip); split halves
                # pipeline the final store latency
                fs32 = outp.tile([128, 512], F32, name="fs32", bufs=2)
                w = 512 // split
                for s in range(split):
                    nc.scalar.activation(
                        fs32[:, s * w : (s + 1) * w],
                        psy[oc][:, s * w : (s + 1) * w],
                        AF.Relu,
                        bias=bmisc_sb[:, 2 + oc : 3 + oc],
                        scale=bmisc_sb[:, oc : oc + 1],
                    )
                    nc.sync.dma_start(
                        feat[
                            128 * oc : 128 * oc + 128,
                            512 * hh + s * w : 512 * hh + (s + 1) * w,
                        ],
                        fs32[:, s * w : (s + 1) * w],
                    )

            # ---- q projection for pass A (cols 640:1152, one 512-wide chunk) ----
            emit_q_chunk(0, 640, NE)

            # ---- streaming + pass A = (br0, h1), pipelined ----
            # per nt: k(x2) -> S+exp(2nt) right away; psv and the second
            # chunk after, so the first exp is never queued behind psv work
            PA_BR, PA_H = 0, 1

            def emit_k(i, xt, c0):
                psk = pskp.tile([16, 512], F32, name="psk", tag="psk")
                for cc in range(2):
                    nc.tensor.matmul(
                        psk[:],
                        wqk_sb[:, 32 + 16 * cc : 48 + 16 * cc],
                        xt[:, cc, :],
                        start=(cc == 0),
                        stop=(cc == 1),
                    )
                nc.vector.tensor_scalar_add(
                    k_r[1 - i][:, c0 : c0 + 512], psk[:], bmisc_sb[0:16, 6:7]
                )

            def emit_v(i, xt, c0):
                psv = psvp.tile([128, 512], F32, name="psv")
                for s4 in range(4):
                    for cc in range(2):
                        nc.tensor.matmul(
                            psv[:, 128 * s4 : 128 * s4 + 128],
                            xt[:, cc, 128 * s4 : 128 * s4 + 128],
                            wv_sb[:, 128 * cc : 128 * cc + 128],
                            start=(cc == 0),
                            stop=(cc == 1),
                            skip_group_check=True,
                        )
                # fold bv into vT: softmax weights sum to 1 (gpsimd can't
                # read PSUM, so this must be DVE)
                nc.vector.tensor_scalar_add(
                    vT[i][:, c0 : c0 + 512], psv[:], bmisc_sb[:, 4:5]
                )

            # DRAM views mapping both channel halves into one DMA:
            # [128 part, 2 halves, 512 cols]
            xf2 = [x.rearrange("(two p) n -> p two n", two=2) for x in xf]

            prev_et = None
            for nt in range(8):
                c0 = nt * 512
                xts = {}
                for i in (1, 0):
                    xt = projp.tile([128, 2, 512], BF16, name="xt", bufs=6)
                    nc.sync.dma_start(xt[:], xf2[i][:, :, c0 : c0 + 512])
                    xts[i] = xt

                emit_k(1, xts[1], c0)
                et = emit_S_exp(PA_BR, PA_H, sstpA, 2 * nt)
                if prev_et is not None:
                    emit_AVden(PA_BR, PA_H, avA, denA, prev_et, 2 * nt - 1)
                prev_et = et
                emit_v(1, xts[1], c0)
                emit_k(0, xts[0], c0)

                # deferred xe loads via SWDGE, one ~0.9us chunk per nt; the
                # gate copy writes into the DMA's destination (WAW dep) so
                # its transfer waits for this nt's k chunk
                kg = k_r[0][0:1, c0 + 511 : c0 + 512]
                if nt == 1:
                    nc.gpsimd.tensor_copy(xe_sb[1][0:1, 0:1], kg)
                    nc.gpsimd.dma_start(
                        xe_v[1][:, 0:1, :], xe_dv[1][:, 0:1, :]
                    )
                if nt == 2:
                    nc.gpsimd.tensor_copy(xe_sb[1][0:1, NE : NE + 1], kg)
                    nc.gpsimd.dma_start(
                        xe_v[1][:, 1:2, :], xe_dv[1][:, 1:2, :]
                    )
                if nt == 3:
                    nc.gpsimd.tensor_copy(xe_sb[0][0:1, 0:1], kg)
                    nc.gpsimd.dma_start(
                        xe_v[0][:, :, 0:640], xe_dv[0][:, :, 0:640]
                    )

                et = emit_S_exp(PA_BR, PA_H, sstpA, 2 * nt + 1)
                emit_v(0, xts[0], c0)
                emit_AVden(PA_BR, PA_H, avA, denA, prev_et, 2 * nt)
                prev_et = et
                # pass-2's q chunks: in-loop so they're ready at the boundary
                if nt == 6:
                    emit_q_chunk(0, 0, 384)
                if nt == 7:
                    emit_q_chunk(0, 384, 640)
            # q projections for the later passes + concat residuals: emitted
            # after the stream so the scheduler runs them in pass-A DVE slack
            emit_residual(0, 0)
            emit_residual(0, 1)
            emit_q_chunk(1, 0, 384, sstpA)
            emit_q_chunk(1, 384, 640, sstpA)
            emit_q_chunk(1, 640, NE, sstpA)
            emit_residual(1, 0)
            emit_residual(1, 1)
            # conv weights + mask via SWDGE, gated (WAW) on the last k chunk
            # so their transfers land after the xt stream (DMA engines idle)
            nc.gpsimd.tensor_copy(wcat_sb[0:1, 0:1], k_r[0][0:1, 4095:4096])
            nc.gpsimd.dma_start(wcat_sb[:], wcatd[:])
            nc.gpsimd.tensor_copy(mask_sb[0:1, 0:1], k_r[0][0:1, 4095:4096])
            nc.gpsimd.dma_start(mask_sb[:], maskd[:])
            emit_AVden(PA_BR, PA_H, avA, denA, prev_et, 15)

            # free streaming PSUM, open the h0-sized s_t ring (4 banks)
            cm_psvp.__exit__(None, None, None)
            cm_pskp.__exit__(None, None, None)
            cm_sstp = tc.tile_pool(name="sstp", bufs=2, space="PSUM", side="left")
            sstp = cm_sstp.__enter__()

            # ---- pass 2 = (br0, h0): first 2 pairs, then pass-A normalize ----
            et2 = [emit_S_exp(0, 0, sstp, 0), emit_S_exp(0, 0, sstp, 1)]
            emit_normalize(PA_BR, PA_H, avA, denA, sstpA)
            emit_store(PA_BR, PA_H)
            cm_denpA.__exit__(None, None, None)
            cm_avpA.__exit__(None, None, None)
            cm_sstpA.__exit__(None, None, None)

            cm_avp0 = tc.tile_pool(name="avp0", bufs=1, space="PSUM", side="right")
            avp0 = cm_avp0.__enter__()
            cm_denp0 = tc.tile_pool(name="denp0", bufs=1, space="PSUM", side="right")
            denp0 = cm_denp0.__enter__()
            jch0 = _jchunks(HLEN[0])
            av0t = avp0.tile([128, 640], F32, name="av0t")
            den0t = denp0.tile([16, 640], F32, name="den0t")
            av0 = [av0t[:, jo : jo + jl] for jo, jl in jch0]
            den0 = [den0t[:, jo : jo + jl] for jo, jl in jch0]

            pend = [(et2[0], 0), (et2[1], 1)]
            for pair in range(2, 16):
                pend.append((emit_S_exp(0, 0, sstp, pair), pair))
                if len(pend) > 2:
                    e, p = pend.pop(0)
                    emit_AVden(0, 0, av0, den0, e, p)
            for e, p in pend:
                emit_AVden(0, 0, av0, den0, e, p)

            # ---- pass 3 = (br1, h0): 4-pair prologue (keeps the previous
            # normalize's bb ring slot from stalling S), then pass-2 normalize ----
            et3 = [emit_S_exp(1, 0, sstp, p) for p in range(4)]
            emit_normalize(0, 0, av0, den0, sstp)
            emit_store(0, 0)
            for p in range(2):
                emit_AVden(1, 0, av0, den0, et3[p], p)
            pend = [(et3[2], 2), (et3[3], 3)]
            # upper-channel spad halves need only residuals+mask: emit early
            # (after the AVden catch-up so DVE prioritizes the attn path)
            emit_spad(0, 1)
            emit_spad(1, 1)
            # precompute out_e0 + x2e for the tail's spad (shortens the final
            # normalize -> spad -> conv chain by one add)
            pre_t = outp.tile([128, 512], F32, name="pre_t")
            nc.vector.tensor_add(
                pre_t[:], out_e[0][:, 640:NE], xe_sb[1][:, 640:NE]
            )
            for pair in range(4, 16):
                pend.append((emit_S_exp(1, 0, sstp, pair), pair))
                if len(pend) > 2:
                    e, p = pend.pop(0)
                    emit_AVden(1, 0, av0, den0, e, p)
            for e, p in pend:
                emit_AVden(1, 0, av0, den0, e, p)

            # ---- pass 4 = (br1, h1): first 2 pairs, then pass-3 normalize,
            # spad0, pool swap; conv section 0 interleaved (2 jobs/pair) ----
            et4 = [emit_S_exp(1, 1, sstp, p) for p in range(4)]
            emit_normalize(1, 0, av0, den0, sstp)
            emit_store(1, 0)
            emit_spad(0, 0)
            cm_denp0.__exit__(None, None, None)
            cm_avp0.__exit__(None, None, None)

            cm_psyp = tc.tile_pool(name="psyp", bufs=2, space="PSUM", side="right")
            psyp = cm_psyp.__enter__()
            cm_avp1 = tc.tile_pool(name="avp1", bufs=1, space="PSUM", side="right")
            avp1 = cm_avp1.__enter__()
            cm_denp1 = tc.tile_pool(name="denp1", bufs=1, space="PSUM", side="right")
            denp1 = cm_denp1.__enter__()
            av1 = [avp1.tile([128, 512], F32, name="av1t")]
            den1 = [denp1.tile([16, 512], F32, name="den1t")]
            psy0 = [psyp.tile([128, 512], F32, name=f"psy0_{oc}", tag="psy") for oc in range(2)]

            for p in range(2):
                emit_AVden(1, 1, av1, den1, et4[p], p)
            pend = [(et4[2], 2), (et4[3], 3)]
            ck = 0  # conv job cursor
            for pair in range(4, 16):
                pend.append((emit_S_exp(1, 1, sstp, pair), pair))
                if len(pend) > 2:
                    e, p = pend.pop(0)
                    emit_AVden(1, 1, av1, den1, e, p)
                for _ in range(2):
                    if ck < 24:
                        emit_conv_mm(0, psy0, ck)
                        ck += 1
                        if ck == 18:
                            emit_conv_out(0, psy0, 0)
            for e, p in pend:
                emit_AVden(1, 1, av1, den1, e, p)

            # ---- tail: remaining conv0 jobs keep PE busy during normalize;
            # conv1's cc=1 taps (early spad) overlap the rest of it ----
            while ck < 36:
                emit_conv_mm(0, psy0, ck)
                ck += 1
            emit_conv_out(0, psy0, 1)

            # custom tail normalize for (br1,h1): spad's sm comes from
            # attn_t + pre_t, off the out_e store path
            recip_f = natp.tile([1, 512], F32, name="recip_f")
            nc.vector.reciprocal(recip_f[:], den1[0][0:1, :])
            recip_b = natp.tile([1, 512], BF16, name="recip_b")
            nc.vector.tensor_scalar_mul(recip_b[:], recip_f[:], gamma)
            bbt = sstp.tile([128, 512], F32, name="bb", tag="s_t")
            nc.tensor.matmul(
                bbt[:], ones1b[0:1, :], recip_b[0:1, :], start=True, stop=True
            )
            bcp = natp.tile([128, 512], F32, name="bcp")
            nc.vector.tensor_copy(bcp[:], bbt[:])
            attn_t = natp.tile([128, 512], F32, name="attn_t")
            nc.vector.tensor_mul(attn_t[:], av1[0][:], bcp[:])
            smt = outp.tile([128, 512], BF16, name="smt")
            nc.vector.tensor_add(smt[:], attn_t[:], pre_t[:])
            nc.vector.tensor_mul(
                spad[0][:, 10:18, 1:65],
                smt[:].rearrange("p (r c) -> p r c", r=8),
                mask_sb[:, 640:NE].rearrange("p (r c) -> p r c", r=8),
            )
            nc.vector.tensor_add(
                out_e[1][:, 640:NE], attn_t[:], xe_sb[1][:, 640:NE]
            )
            emit_store(1, 1)
            cm_denp1.__exit__(None, None, None)
            cm_avp1.__exit__(None, None, None)

            psy1 = [psyp.tile([128, 512], F32, name=f"psy1_{oc}", tag="psy") for oc in range(2)]
            for oc in range(2):
                for t in range(9):
                    emit_conv_mm2(1, psy1, oc, t, 1, start=(t == 0), stop=False)
            for oc in range(2):
                for t in range(9):
                    emit_conv_mm2(1, psy1, oc, t, 0, start=False, stop=(t == 8))
                emit_conv_out(1, psy1, oc, split=2)

            cm_psyp.__exit__(None, None, None)
            cm_sstp.__exit__(None, None, None)

    nc.compile()
    return nc


def _prep_inputs(input1, input2, Wq, bq, Wk, bk, Wv, bv, gamma, Wcat, bn_gamma, bn_beta):
    f32 = np.float32
    bf16 = ml_dtypes.bfloat16
    x1 = np.asarray(input1, f32).reshape(B, C, NW)
    x2 = np.asarray(input2, f32).reshape(B, C, NW)
    x1b = np.ascontiguousarray(x1.astype(bf16))
    x2b = np.ascontiguousarray(x2.astype(bf16))
    Wq, Wk, Wv = (np.asarray(w, f32) for w in (Wq, Wk, Wv))
    Wcat = np.asarray(Wcat, f32)

    wqk = np.zeros((128, 64), f32)
    wqk[:, 0:16] = Wq.T[0:128]
    wqk[:, 16:32] = Wq.T[128:256]
    wqk[:, 32:48] = Wk.T[0:128]
    wqk[:, 48:64] = Wk.T[128:256]
    wqk = wqk.astype(bf16)

    wv2 = np.zeros((128, 256), f32)
    wv2[:, 0:128] = Wv.T[0:128]
    wv2[:, 128:256] = Wv.T[128:256]
    wv2 = wv2.astype(bf16)

    # [t, c, o] -> per c-chunk [128, 9*256]
    Wt = Wcat.transpose(2, 3, 1, 0).reshape(9, 256, 256)
    wcat2 = np.zeros((128, 4608), f32)
    for cc in range(2):
        wcat2[:, 2304 * cc : 2304 * (cc + 1)] = (
            Wt[:, 128 * cc : 128 * (cc + 1), :].transpose(1, 0, 2).reshape(128, 2304)
        )
    wcat2 = wcat2.astype(bf16)

    bnscale = (np.asarray(bn_gamma, f32) / np.sqrt(f32(1.0) + f32(BN_EPS))).astype(f32)
    bnb = np.asarray(bn_beta, f32)
    bmisc = np.zeros((128, 8), f32)
    bmisc[:, 0] = bnscale[0:128]
    bmisc[:, 1] = bnscale[128:256]
    bmisc[:, 2] = bnb[0:128]
    bmisc[:, 3] = bnb[128:256]
    bmisc[:, 4] = np.asarray(bv, f32)
    bmisc[0:16, 5] = np.asarray(bq, f32)
    bmisc[0:16, 6] = np.asarray(bk, f32)

    in_maps = []
    for core in range(N_CORES):
        b, rg = core // 4, core % 4
        r0 = 16 * rg
        lo = r0 - 1

        def ext(x):
            e = np.zeros((C, RE, W), f32)
            g0, g1 = max(0, lo), min(H, lo + RE)
            e[:, g0 - lo : g1 - lo, :] = x[b].reshape(C, H, W)[:, g0:g1, :]
            return np.ascontiguousarray(e.reshape(C, NE).astype(bf16))

        rows = np.ones(RE, f32)
        if rg == 0:
            rows[0] = 0.0
        if rg == 3:
            rows[RE - 1] = 0.0
        mask = np.broadcast_to(np.repeat(rows, W)[None, :], (128, NE)).astype(bf16)

        in_maps.append(
            {
                "x1f": np.ascontiguousarray(x1b[b]),
                "x2f": np.ascontiguousarray(x2b[b]),
                "x1e": ext(x1),
                "x2e": ext(x2),
                "maskd": np.ascontiguousarray(mask),
                "wqkd": wqk,
                "wvd": wv2,
                "wcatd": wcat2,
                "bmiscd": bmisc,
            }
        )
    return in_maps


def _assemble(results):
    f32 = np.float32
    feat_sum = np.empty((B, C, H, W), f32)
    out1 = np.empty((B, C, H, W), f32)
    out2 = np.empty((B, C, H, W), f32)
    for core in range(N_CORES):
        b, rg = core // 4, core % 4
        r0 = 16 * rg
        r = results[core]
        out1[b, :, r0 : r0 + 16] = np.asarray(r["o1"], f32).reshape(C, 16, W)
        out2[b, :, r0 : r0 + 16] = np.asarray(r["o2"], f32).reshape(C, 16, W)
        feat_sum[b, :, r0 : r0 + 16] = np.asarray(r["feat"], f32).reshape(C, 16, W)
    return feat_sum, out1, out2


def _get_program(gamma: float):
    if gamma not in _PROG_CACHE:
        _PROG_CACHE[gamma] = _build_program(gamma)
    return _PROG_CACHE[gamma]


def kernel(input1, input2, Wq, bq, Wk, bk, Wv, bv, gamma, Wcat, bn_gamma, bn_beta):
    g = float(np.asarray(gamma).reshape(-1)[0])
    nc = _get_program(g)
    in_maps = _prep_inputs(
        input1, input2, Wq, bq, Wk, bk, Wv, bv, gamma, Wcat, bn_gamma, bn_beta
    )
    res = run_bass_kernel_spmd(nc, in_maps, core_ids=list(range(N_CORES)))
    return _assemble(res.results)


def run_traced(inputs):
    """For test.py: run, return (outputs, exec_time_ns)."""
    g = float(np.asarray(inputs["gamma"]).reshape(-1)[0])
    nc = _get_program(g)
    in_maps = _prep_inputs(**inputs)
    res = run_bass_kernel_spmd(nc, in_maps, core_ids=list(range(N_CORES)))
    return _assemble(res.results), res.exec_time_ns


# revision 73
# speedup vs baseline: 1.0007x; 1.0007x over previous
"""Trainium2 Bass kernel for nn_CrossAtt (cross-attention + concat + residual +
3x3 conv + BN + ReLU), data-parallel over (batch, row-group) across 8 cores.

Sharding: core i -> batch b = i//4, row-group rg = i%4 (16 output rows each).
Each core computes both attention branches for an 18-row extended window
(16 rows + 1 halo row each side, zero-masked at image edges), the fused
residual/concat outputs, and the 3x3 conv + BN + ReLU on its 16 rows.

v3 design (cost-model driven, Act-engine pacing; 150.9us -> 116.6us):
- all matmul operands bf16; exp tiles + vT fp8e4 -> AV/den matmuls DoubleRow
- 4 passes: A=(br0,h1) overlapped with projection streaming, (br0,h0),
  (br1,h0), (br1,h1)+conv section 0 interleaved; conv section 1 in tail
- pipelined emission: AVden(p) delayed 1-2 pairs behind S(p) so exp never
  queues behind an AVden waiting on a previous exp
- pass-boundary interleaving: next pass's first 2-4 pairs of S+exp emitted
  before the previous pass's normalize; PSUM pools swapped exactly between
- normalize: gamma folded into the reciprocal broadcast, bv folded into the
  vT copy (softmax weights sum to 1), residual concat-half emitted early
- DMA discipline: a dma_start holds its queue's SEQ until its serial HWDGE
  slot completes (~1.3us), so the Act queue carries no DMAs; xt stream =
  one merged DMA per (nt,image) on the SP queue; x2e/xeh0/wcat/mask go via
  SWDGE with WAW gate-copies pacing their transfers into stream slack
- tail: conv1 upper-channel taps (early spad) overlap the final normalize;
  tiny t=0 Exp hoists the ACT_TABLE_LOAD; BN+ReLU writes f32 directly,
  split stores pipeline the last DMA
- PSUM: head 8 = psk2+psv2+sstA2+avA1+denA1; h0 8 = sst4+av2+den2;
  pass4 8 = sst4+av1+den1+psy2
"""

import sys

sys.path.insert(0, "/opt/trn_rl_repo")

import numpy as np
import ml_dtypes

import concourse.bacc as bacc
import concourse.tile as tile
from concourse import mybir
from concourse.bass_utils import run_bass_kernel_spmd

F32 = mybir.dt.float32
BF16 = mybir.dt.bfloat16
FP8 = mybir.dt.float8e4
AF = mybir.ActivationFunctionType
ALU = mybir.AluOpType
DR = mybir.MatmulPerfMode.DoubleRow

B, C, H, W = 2, 256, 64, 64
NW = H * W  # 4096 key/value positions
RE = 18  # extended rows per core
NE = RE * W  # 1152 query positions per core
D_QK, D_V = 16, 128
N_CORES = 8
BN_EPS = 1e-5
HLEN = (640, 512)  # window halves: rows 0..9 | rows 10..17
HOFF = (0, 640)
EXP_BIAS = -3.5  # exp(S/4 - 3.5): keeps fp8 weights < 240; cancels in softmax

_PROG_CACHE: dict = {}


def _jchunks(hlen):
    # bank-aligned output chunks (PSUM bank = 512 f32)
    return [(0, 512), (512, hlen - 512)] if hlen > 512 else [(0, hlen)]


def _build_program(gamma: float):
    nc = bacc.Bacc("TRN2", target_bir_lowering=False, debug=False, num_devices=N_CORES)

    def din(name, shape, dt=BF16):
        return nc.dram_tensor(name, shape, dt, kind="ExternalInput").ap()

    def dout(name, shape):
        return nc.dram_tensor(name, shape, F32, kind="ExternalOutput").ap()

    x1f = din("x1f", [C, NW])
    x2f = din("x2f", [C, NW])
    x1e = din("x1e", [C, NE])
    x2e = din("x2e", [C, NE])
    maskd = din("maskd", [128, NE])
    wqkd = din("wqkd", [128, 64])
    wvd = din("wvd", [128, 256])
    wcatd = din("wcatd", [128, 4608])
    # packed small params: [:,0:4]=bn scale/beta (lo|hi), [:,4]=bv,
    # [0:16,5]=bq, [0:16,6]=bk
    bmiscd = din("bmiscd", [128, 8], F32)
    o1 = dout("o1", [C, 1024])
    o2 = dout("o2", [C, 1024])
    feat = dout("feat", [C, 1024])

    xf = [x1f, x2f]
    xe_d = [x1e, x2e]
    od = [o1, o2]

    with tile.TileContext(nc) as tc:
        with (
            tc.tile_pool(name="constp", bufs=1) as constp,
            tc.tile_pool(name="projp", bufs=1) as projp,
            tc.tile_pool(name="outp", bufs=1) as outp,
            tc.tile_pool(name="etp", bufs=8) as etp,
            tc.tile_pool(name="natp", bufs=2) as natp,
        ):
            # ---- SBUF tiles ----
            wqk_sb = constp.tile([128, 64], BF16, name="wqk_sb")
            wv_sb = constp.tile([128, 256], BF16, name="wv_sb")
            bmisc_sb = constp.tile([128, 8], F32, name="bmisc_sb")
            mask_sb = constp.tile([128, NE], BF16, name="mask_sb")
            wcat_sb = constp.tile([128, 4608], BF16, name="wcat_sb")
            xe_sb = [
                constp.tile([128, 2 * NE], BF16, name=f"xe_sb{i}") for i in range(2)
            ]
            ebias = constp.tile([128, 1], F32, name="ebias")
            ones8 = constp.tile([128, 32], FP8, name="ones8")
            ones1b = constp.tile([1, 128], BF16, name="ones1b")

            k_r = [projp.tile([16, NW], BF16, name=f"k_r{i}") for i in range(2)]
            q_r = [projp.tile([16, NE], BF16, name=f"q_r{i}") for i in range(2)]
            vT = [projp.tile([128, NW], FP8, name=f"vT{i}") for i in range(2)]

            spad = []
            for cc in range(2):
                sp = outp.tile([128, RE, 66], BF16, name=f"spad{cc}")
                spad.append(sp)
            out_e = [
                outp.tile([128, 2 * NE], F32, name=f"out_e{br}") for br in range(2)
            ]

            # ---- head DMAs ----
            # HWDGE queues carry ONLY the critical path (x1e + xt stream +
            # stores): a dma_start holds its engine's SEQ until its HWDGE
            # slot completes (~1.3us), so everything else goes via SWDGE
            # (Pool engine), gated to avoid DMA_ENGINES contention.
            nc.sync.dma_start(wqk_sb[:], wqkd[:])
            # x1e h1-half first (one merged DMA over both channel planes):
            # pass A's q projection (cols 640:1152) gates the first S matmul
            xe_v = [
                t[:].rearrange("p (two n) -> p two n", two=2) for t in xe_sb
            ]
            xe_dv = [
                x.rearrange("(two p) n -> p two n", two=2) for x in xe_d
            ]
            nc.sync.dma_start(xe_v[0][:, :, 640:NE], xe_dv[0][:, :, 640:NE])

            # Pool/SWDGE queue: small consts now; x2e/wcat/mask deferred to
            # nt-loop hooks behind gate copies that write INTO the DMA's
            # destination tile (WAW dep - scheduler can't hoist the DMA)
            nc.gpsimd.dma_start(bmisc_sb[:], bmiscd[:])
            nc.gpsimd.dma_start(wv_sb[:], wvd[:])
            nc.gpsimd.memset(ebias[:], EXP_BIAS)
            nc.gpsimd.memset(ones8[:], 1.0)
            nc.gpsimd.memset(ones1b[:], 1.0)
            for cc in range(2):
                nc.gpsimd.memset(spad[cc][:], 0.0)

            onesv = ones8[:].rearrange("p (two m) -> p two m", two=2)

            # ---- PSUM pools (head phase) ----
            # two independent stacks: left = streaming/s_t ring, right = av/den
            cm_pskp = tc.tile_pool(name="pskp", bufs=2, space="PSUM", side="left")
            pskp = cm_pskp.__enter__()
            cm_psvp = tc.tile_pool(name="psvp", bufs=2, space="PSUM", side="left")
            psvp = cm_psvp.__enter__()
            cm_sstpA = tc.tile_pool(name="sstpA", bufs=2, space="PSUM", side="right")
            sstpA = cm_sstpA.__enter__()
            cm_avpA = tc.tile_pool(name="avpA", bufs=1, space="PSUM", side="right")
            avpA = cm_avpA.__enter__()
            cm_denpA = tc.tile_pool(name="denpA", bufs=1, space="PSUM", side="right")
            denpA = cm_denpA.__enter__()

            avA = [avpA.tile([128, 512], F32, name="avA0")]
            denA = [denpA.tile([16, 512], F32, name="denA0")]

            # tiny dummy Exp at t=0: pulls the ACT_TABLE_LOAD off the first
            # real exp's critical path
            dummy = constp.tile([1, 1], F32, name="dummy")
            nc.scalar.activation(dummy[:], ebias[0:1, 0:1], AF.Exp)

            # ---- emission helpers ----
            def emit_q_chunk(i, q0, q1, pool=None):
                # q_r[i] cols q0:q1 (<=512 wide, PSUM bank); post-stream
                # chunks ride the sstpA ring so pskp can close early
                psq = (pool or pskp).tile(
                    [16, q1 - q0], F32, name="psk",
                    tag="psk" if pool is None else "s_t",
                )
                for cc in range(2):
                    nc.tensor.matmul(
                        psq[:],
                        wqk_sb[:, 16 * cc : 16 * cc + 16],
                        xe_sb[i][:, NE * cc + q0 : NE * cc + q1],
                        start=(cc == 0),
                        stop=(cc == 1),
                    )
                nc.vector.tensor_scalar_add(
                    q_r[i][:, q0:q1], psq[:], bmisc_sb[0:16, 5:6]
                )

            def emit_residual(br, h):
                # concat upper half: out_e[:,NE:2NE] = gamma*x_lo + x_hi
                c0, c1 = (0, 640) if h == 0 else (640, NE)
                nc.vector.scalar_tensor_tensor(
                    out_e[br][:, NE + c0 : NE + c1],
                    xe_sb[br][:, c0:c1],
                    gamma,
                    xe_sb[br][:, NE + c0 : NE + c1],
                    ALU.mult,
                    ALU.add,
                )

            def emit_S_exp(br, h, sstp_, pair):
                hlen, hoff = HLEN[h], HOFF[h]
                jch = _jchunks(hlen)
                et_t = etp.tile([128, 2 * hlen], FP8, name="et")
                for par in range(2):
                    mi = 2 * pair + par
                    s_t = sstp_.tile([128, hlen], F32, name="s_t", tag="s_t")
                    for jo, jl in jch:
                        nc.tensor.matmul(
                            s_t[:, jo : jo + jl],
                            k_r[br][:, mi * 128 : mi * 128 + 128],
                            q_r[br][:, hoff + jo : hoff + jo + jl],
                            start=True,
                            stop=True,
                        )
                    nc.scalar.activation(
                        et_t[:, par * hlen : (par + 1) * hlen],
                        s_t[:],
                        AF.Exp,
                        bias=ebias[:],
                        scale=0.25,
                    )
                return et_t

            def emit_AVden(br, h, av, den, et_t, pair):
                hlen = HLEN[h]
                jch = _jchunks(hlen)
                etv = et_t[:].rearrange("p (two n) -> p two n", two=2)
                vv = vT[br][:, pair * 256 : (pair + 1) * 256].rearrange(
                    "p (two m) -> p two m", two=2
                )
                for j, (jo, jl) in enumerate(jch):
                    nc.tensor.matmul(
                        av[j][:],
                        vv,
                        etv[:, :, jo : jo + jl],
                        start=(pair == 0),
                        stop=(pair == 15),
                        perf_mode=DR,
                        skip_group_check=True,
                    )
                    nc.tensor.matmul(
                        den[j][:],
                        onesv[:, :, 0:16],
                        etv[:, :, jo : jo + jl],
                        start=(pair == 0),
                        stop=(pair == 15),
                        perf_mode=DR,
                        skip_group_check=True,
                    )

            def emit_normalize(br, h, av, den, bb_pool):
                hlen, hoff = HLEN[h], HOFF[h]
                jch = _jchunks(hlen)
                recip_f = natp.tile([1, hlen], F32, name="recip_f")
                for j, (jo, jl) in enumerate(jch):
                    nc.vector.reciprocal(recip_f[0:1, jo : jo + jl], den[j][0:1, :])
                recip_b = natp.tile([1, hlen], BF16, name="recip_b")
                nc.vector.tensor_scalar_mul(recip_b[:], recip_f[:], gamma)
                bb = bb_pool.tile([128, hlen], F32, name="bb", tag="s_t")
                for jo, jl in jch:
                    nc.tensor.matmul(
                        bb[:, jo : jo + jl],
                        ones1b[0:1, :],
                        recip_b[0:1, jo : jo + jl],
                        start=True,
                        stop=True,
                    )
                for j, (jo, jl) in enumerate(jch):
                    bcp = natp.tile([128, jl], F32, name="bcp")
                    nc.vector.tensor_copy(bcp[:], bb[:, jo : jo + jl])
                    attn_t = natp.tile([128, jl], F32, name="attn_t")
                    nc.vector.tensor_mul(attn_t[:], av[j][:], bcp[:])
                    c0 = hoff + jo
                    nc.vector.tensor_add(
                        out_e[br][:, c0 : c0 + jl],
                        attn_t[:],
                        xe_sb[br][:, c0 : c0 + jl],
                    )

            def emit_store(br, h):
                if h == 0:
                    nc.sync.dma_start(od[br][0:128, 0:576], out_e[br][:, 64:640])
                    nc.sync.dma_start(
                        od[br][128:256, 0:576], out_e[br][:, NE + 64 : NE + 640]
                    )
                else:
                    nc.sync.dma_start(od[br][0:128, 576:1024], out_e[br][:, 640:1088])
                    nc.sync.dma_start(
                        od[br][128:256, 576:1024], out_e[br][:, NE + 640 : NE + 1088]
                    )

            def emit_spad(sec, cc):
                # sec 0: rows 0..9 <- cols 0:640 ; sec 1: rows 10..17 <- cols 640:1152
                # cc=1 (upper channels) depends only on the early residuals
                r0, r1 = (0, 10) if sec == 0 else (10, 18)
                c0, c1 = (0, 640) if sec == 0 else (640, 1152)
                nr = r1 - r0
                sm = outp.tile([128, 640], BF16, name="sm", bufs=2)
                nc.vector.tensor_add(
                    sm[:, 0 : c1 - c0],
                    out_e[0][:, NE * cc + c0 : NE * cc + c1],
                    out_e[1][:, NE * cc + c0 : NE * cc + c1],
                )
                nc.vector.tensor_mul(
                    spad[cc][:, r0:r1, 1:65],
                    sm[:, 0 : c1 - c0].rearrange("p (r c) -> p r c", r=nr),
                    mask_sb[:, c0:c1].rearrange("p (r c) -> p r c", r=nr),
                )

            conv_jobs = [
                (oc, t, cc) for oc in range(2) for t in range(9) for cc in range(2)
            ]

            def emit_conv_mm2(hh, psy, oc, t, cc, start, stop):
                dy, dx = t // 3, t % 3
                nc.tensor.matmul(
                    psy[oc][:],
                    wcat_sb[
                        :,
                        2304 * cc + 256 * t + 128 * oc : 2304 * cc
                        + 256 * t
                        + 128 * oc
                        + 128,
                    ],
                    spad[cc][:, 8 * hh + dy : 8 * hh + dy + 8, dx : dx + 64],
                    start=start,
                    stop=stop,
                    skip_group_check=True,
                )

            def emit_conv_mm(hh, psy, k):
                oc, t, cc = conv_jobs[k]
                emit_conv_mm2(
                    hh, psy, oc, t, cc, start=(t == 0 and cc == 0),
                    stop=(t == 8 and cc == 1),
                )

            def emit_conv_out(hh, psy, oc, split=1):
                # BN+ReLU straight to f32 (no bf16 round-trip); split halves
                # pipeline the final store latency
                fs32 = outp.tile([128, 512], F32, name="fs32", bufs=2)
                w = 512 // split
                for s in range(split):
                    nc.scalar.activation(
                        fs32[:, s * w : (s + 1) * w],
                        psy[oc][:, s * w : (s + 1) * w],
                        AF.Relu,
                        bias=bmisc_sb[:, 2 + oc : 3 + oc],
                        scale=bmisc_sb[:, oc : oc + 1],
                    )
                    nc.sync.dma_start(
                        feat[
                            128 * oc : 128 * oc + 128,
                            512 * hh + s * w : 512 * hh + (s + 1) * w,
                        ],
                        fs32[:, s * w : (s + 1) * w],
                    )

            # ---- q projection for pass A (cols 640:1152, one 512-wide chunk) ----
            emit_q_chunk(0, 640, NE)

            # ---- streaming + pass A = (br0, h1), pipelined ----
            # per nt: k(x2) -> S+exp(2nt) right away; psv and the second
            # chunk after, so the first exp is never queued behind psv work
            PA_BR, PA_H = 0, 1

            def emit_k(i, xt, c0):
                psk = pskp.tile([16, 512], F32, name="psk", tag="psk")
                for cc in range(2):
                    nc.tensor.matmul(
                        psk[:],
                        wqk_sb[:, 32 + 16 * cc : 48 + 16 * cc],
                        xt[:, cc, :],
                        start=(cc == 0),
                        stop=(cc == 1),
                    )
                nc.vector.tensor_scalar_add(
                    k_r[1 - i][:, c0 : c0 + 512], psk[:], bmisc_sb[0:16, 6:7]
                )

            def emit_v(i, xt, c0):
                psv = psvp.tile([128, 512], F32, name="psv")
                for s4 in range(4):
                    for cc in range(2):
                        nc.tensor.matmul(
                            psv[:, 128 * s4 : 128 * s4 + 128],
                            xt[:, cc, 128 * s4 : 128 * s4 + 128],
                            wv_sb[:, 128 * cc : 128 * cc + 128],
                            start=(cc == 0),
                            stop=(cc == 1),
                            skip_group_check=True,
                        )
                # fold bv into vT: softmax weights sum to 1 (gpsimd can't
                # read PSUM, so this must be DVE)
                nc.vector.tensor_scalar_add(
                    vT[i][:, c0 : c0 + 512], psv[:], bmisc_sb[:, 4:5]
                )

            # DRAM views mapping both channel halves into one DMA:
            # [128 part, 2 halves, 512 cols]
            xf2 = [x.rearrange("(two p) n -> p two n", two=2) for x in xf]

            prev_et = None
            for nt in range(8):
                c0 = nt * 512
                xts = {}
                for i in (1, 0):
                    xt = projp.tile([128, 2, 512], BF16, name="xt", bufs=6)
                    nc.sync.dma_start(xt[:], xf2[i][:, :, c0 : c0 + 512])
                    xts[i] = xt

                emit_k(1, xts[1], c0)
                et = emit_S_exp(PA_BR, PA_H, sstpA, 2 * nt)
                if prev_et is not None:
                    emit_AVden(PA_BR, PA_H, avA, denA, prev_et, 2 * nt - 1)
                prev_et = et
                emit_v(1, xts[1], c0)
                emit_k(0, xts[0], c0)

                # deferred xe loads via SWDGE, one ~0.9us chunk per nt; the
                # gate copy writes into the DMA's destination (WAW dep) so
                # its transfer waits for this nt's k chunk
                kg = k_r[0][0:1, c0 + 511 : c0 + 512]
                if nt == 1:
                    nc.gpsimd.tensor_copy(xe_sb[1][0:1, 0:1], kg)
                    nc.gpsimd.dma_start(
                        xe_v[1][:, 0:1, :], xe_dv[1][:, 0:1, :]
                    )
                if nt == 2:
                    nc.gpsimd.tensor_copy(xe_sb[1][0:1, NE : NE + 1], kg)
                    nc.gpsimd.dma_start(
                        xe_v[1][:, 1:2, :], xe_dv[1][:, 1:2, :]
                    )
                if nt == 3:
                    nc.gpsimd.tensor_copy(xe_sb[0][0:1, 0:1], kg)
                    nc.gpsimd.dma_start(
                        xe_v[0][:, :, 0:640], xe_dv[0][:, :, 0:640]
                    )

                et = emit_S_exp(PA_BR, PA_H, sstpA, 2 * nt + 1)
                emit_v(0, xts[0], c0)
                emit_AVden(PA_BR, PA_H, avA, denA, prev_et, 2 * nt)
                prev_et = et
                # pass-2's q chunks: in-loop so they're ready at the boundary
                if nt == 6:
                    emit_q_chunk(0, 0, 384)
                if nt == 7:
                    emit_q_chunk(0, 384, 640)
            # q projections for the later passes + concat residuals: emitted
            # after the stream so the scheduler runs them in pass-A DVE slack
            emit_residual(0, 0)
            emit_residual(0, 1)
            emit_q_chunk(1, 0, 384, sstpA)
            emit_q_chunk(1, 384, 640, sstpA)
            emit_q_chunk(1, 640, NE, sstpA)
            emit_residual(1, 0)
            emit_residual(1, 1)
            # conv weights + mask via SWDGE, gated (WAW) on the last k chunk
            # so their transfers land after the xt stream (DMA engines idle)
            nc.gpsimd.tensor_copy(wcat_sb[0:1, 0:1], k_r[0][0:1, 4095:4096])
            nc.gpsimd.dma_start(wcat_sb[:], wcatd[:])
            nc.gpsimd.tensor_copy(mask_sb[0:1, 0:1], k_r[0][0:1, 4095:4096])
            nc.gpsimd.dma_start(mask_sb[:], maskd[:])
            emit_AVden(PA_BR, PA_H, avA, denA, prev_et, 15)

            # free streaming PSUM, open the h0-sized s_t ring (4 banks)
            cm_psvp.__exit__(None, None, None)
            cm_pskp.__exit__(None, None, None)
            cm_sstp = tc.tile_pool(name="sstp", bufs=2, space="PSUM", side="left")
            sstp = cm_sstp.__enter__()

            # ---- pass 2 = (br0, h0): first 2 pairs, then pass-A normalize ----
            et2 = [emit_S_exp(0, 0, sstp, 0), emit_S_exp(0, 0, sstp, 1)]
            emit_normalize(PA_BR, PA_H, avA, denA, sstpA)
            emit_store(PA_BR, PA_H)
            cm_denpA.__exit__(None, None, None)
            cm_avpA.__exit__(None, None, None)
            cm_sstpA.__exit__(None, None, None)

            cm_avp0 = tc.tile_pool(name="avp0", bufs=1, space="PSUM", side="right")
            avp0 = cm_avp0.__enter__()
            cm_denp0 = tc.tile_pool(name="denp0", bufs=1, space="PSUM", side="right")
            denp0 = cm_denp0.__enter__()
            jch0 = _jchunks(HLEN[0])
            av0t = avp0.tile([128, 640], F32, name="av0t")
            den0t = denp0.tile([16, 640], F32, name="den0t")
            av0 = [av0t[:, jo : jo + jl] for jo, jl in jch0]
            den0 = [den0t[:, jo : jo + jl] for jo, jl in jch0]

            pend = [(et2[0], 0), (et2[1], 1)]
            for pair in range(2, 16):
                pend.append((emit_S_exp(0, 0, sstp, pair), pair))
                if len(pend) > 2:
                    e, p = pend.pop(0)
                    emit_AVden(0, 0, av0, den0, e, p)
            for e, p in pend:
                emit_AVden(0, 0, av0, den0, e, p)

            # ---- pass 3 = (br1, h0): 4-pair prologue (keeps the previous
            # normalize's bb ring slot from stalling S), then pass-2 normalize ----
            et3 = [emit_S_exp(1, 0, sstp, p) for p in range(4)]
            emit_normalize(0, 0, av0, den0, sstp)
            emit_store(0, 0)
            for p in range(2):
                emit_AVden(1, 0, av0, den0, et3[p], p)
            pend = [(et3[2], 2), (et3[3], 3)]
            # upper-channel spad halves need only residuals+mask: emit early
            # (after the AVden catch-up so DVE prioritizes the attn path)
            emit_spad(0, 1)
            emit_spad(1, 1)
            # precompute out_e0 + x2e for the tail's spad (shortens the final
            # normalize -> spad -> conv chain by one add)
            pre_t = outp.tile([128, 512], F32, name="pre_t")
            nc.vector.tensor_add(
                pre_t[:], out_e[0][:, 640:NE], xe_sb[1][:, 640:NE]
            )
            for pair in range(4, 16):
                pend.append((emit_S_exp(1, 0, sstp, pair), pair))
                if len(pend) > 2:
                    e, p = pend.pop(0)
                    emit_AVden(1, 0, av0, den0, e, p)
            for e, p in pend:
                emit_AVden(1, 0, av0, den0, e, p)

            # ---- pass 4 = (br1, h1): first 2 pairs, then pass-3 normalize,
            # spad0, pool swap; conv section 0 interleaved (2 jobs/pair) ----
            et4 = [emit_S_exp(1, 1, sstp, p) for p in range(4)]
            emit_normalize(1, 0, av0, den0, sstp)
            emit_store(1, 0)
            emit_spad(0, 0)
            cm_denp0.__exit__(None, None, None)
            cm_avp0.__exit__(None, None, None)

            cm_psyp = tc.tile_pool(name="psyp", bufs=2, space="PSUM", side="right")
            psyp = cm_psyp.__enter__()
            cm_avp1 = tc.tile_pool(name="avp1", bufs=1, space="PSUM", side="right")
            avp1 = cm_avp1.__enter__()
            cm_denp1 = tc.tile_pool(name="denp1", bufs=1, space="PSUM", side="right")
            denp1 = cm_denp1.__enter__()
            av1 = [avp1.tile([128, 512], F32, name="av1t")]
            den1 = [denp1.tile([16, 512], F32, name="den1t")]
            psy0 = [psyp.tile([128, 512], F32, name=f"psy0_{oc}", tag="psy") for oc in range(2)]

            for p in range(2):
                emit_AVden(1, 1, av1, den1, et4[p], p)
            pend = [(et4[2], 2), (et4[3], 3)]
            ck = 0  # conv job cursor
            for pair in range(4, 16):
                pend.append((emit_S_exp(1, 1, sstp, pair), pair))
                if len(pend) > 2:
                    e, p = pend.pop(0)
                    emit_AVden(1, 1, av1, den1, e, p)
                for _ in range(2):
                    if ck < 24:
                        emit_conv_mm(0, psy0, ck)
                        ck += 1
                        if ck == 18:
                            emit_conv_out(0, psy0, 0)
            for e, p in pend:
                emit_AVden(1, 1, av1, den1, e, p)

            # ---- tail: remaining conv0 jobs keep PE busy during normalize;
            # conv1's cc=1 taps (early spad) overlap the rest of it ----
            while ck < 36:
                emit_conv_mm(0, psy0, ck)
                ck += 1
            emit_conv_out(0, psy0, 1)

            # custom tail normalize for (br1,h1): spad's sm comes from
            # attn_t + pre_t, off the out_e store path
            recip_f = natp.tile([1, 512], F32, name="recip_f")
            nc.vector.reciprocal(recip_f[:], den1[0][0:1, :])
            recip_b = natp.tile([1, 512], BF16, name="recip_b")
            nc.vector.tensor_scalar_mul(recip_b[:], recip_f[:], gamma)
            bbt = sstp.tile([128, 512], F32, name="bb", tag="s_t")
            nc.tensor.matmul(
                bbt[:], ones1b[0:1, :], recip_b[0:1, :], start=True, stop=True
            )
            bcp = natp.tile([128, 512], F32, name="bcp")
            nc.vector.tensor_copy(bcp[:], bbt[:])
            attn_t = natp.tile([128, 512], F32, name="attn_t")
            nc.vector.tensor_mul(attn_t[:], av1[0][:], bcp[:])
            smt = outp.tile([128, 512], BF16, name="smt")
            nc.vector.tensor_add(smt[:], attn_t[:], pre_t[:])
            nc.vector.tensor_mul(
                spad[0][:, 10:18, 1:65],
                smt[:].rearrange("p (r c) -> p r c", r=8),
                mask_sb[:, 640:NE].rearrange("p (r c) -> p r c", r=8),
            )
            nc.vector.tensor_add(
                out_e[1][:, 640:NE], attn_t[:], xe_sb[1][:, 640:NE]
            )
            emit_store(1, 1)
            cm_denp1.__exit__(None, None, None)
            cm_avp1.__exit__(None, None, None)

            psy1 = [psyp.tile([128, 512], F32, name=f"psy1_{oc}", tag="psy") for oc in range(2)]
            for oc in range(2):
                for t in range(9):
                    emit_conv_mm2(1, psy1, oc, t, 1, start=(t == 0), stop=False)
            for oc in range(2):
                for t in range(9):
                    emit_conv_mm2(1, psy1, oc, t, 0, start=False, stop=(t == 8))
                emit_conv_out(1, psy1, oc, split=2)

            cm_psyp.__exit__(None, None, None)
            cm_sstp.__exit__(None, None, None)

    nc.compile()
    return nc


def _prep_inputs(input1, input2, Wq, bq, Wk, bk, Wv, bv, gamma, Wcat, bn_gamma, bn_beta):
    f32 = np.float32
    bf16 = ml_dtypes.bfloat16
    x1 = np.asarray(input1, f32).reshape(B, C, NW)
    x2 = np.asarray(input2, f32).reshape(B, C, NW)
    x1b = np.ascontiguousarray(x1.astype(bf16))
    x2b = np.ascontiguousarray(x2.astype(bf16))
    Wq, Wk, Wv = (np.asarray(w, f32) for w in (Wq, Wk, Wv))
    Wcat = np.asarray(Wcat, f32)

    wqk = np.zeros((128, 64), f32)
    wqk[:, 0:16] = Wq.T[0:128]
    wqk[:, 16:32] = Wq.T[128:256]
    wqk[:, 32:48] = Wk.T[0:128]
    wqk[:, 48:64] = Wk.T[128:256]
    wqk = wqk.astype(bf16)

    wv2 = np.zeros((128, 256), f32)
    wv2[:, 0:128] = Wv.T[0:128]
    wv2[:, 128:256] = Wv.T[128:256]
    wv2 = wv2.astype(bf16)

    # [t, c, o] -> per c-chunk [128, 9*256]
    Wt = Wcat.transpose(2, 3, 1, 0).reshape(9, 256, 256)
    wcat2 = np.zeros((128, 4608), f32)
    for cc in range(2):
        wcat2[:, 2304 * cc : 2304 * (cc + 1)] = (
            Wt[:, 128 * cc : 128 * (cc + 1), :].transpose(1, 0, 2).reshape(128, 2304)
        )
    wcat2 = wcat2.astype(bf16)

    bnscale = (np.asarray(bn_gamma, f32) / np.sqrt(f32(1.0) + f32(BN_EPS))).astype(f32)
    bnb = np.asarray(bn_beta, f32)
    bmisc = np.zeros((128, 8), f32)
    bmisc[:, 0] = bnscale[0:128]
    bmisc[:, 1] = bnscale[128:256]
    bmisc[:, 2] = bnb[0:128]
    bmisc[:, 3] = bnb[128:256]
    bmisc[:, 4] = np.asarray(bv, f32)
    bmisc[0:16, 5] = np.asarray(bq, f32)
    bmisc[0:16, 6] = np.asarray(bk, f32)

    in_maps = []
    for core in range(N_CORES):
        b, rg = core // 4, core % 4
        r0 = 16 * rg
        lo = r0 - 1

        def ext(x):
            e = np.zeros((C, RE, W), f32)
            g0, g1 = max(0, lo), min(H, lo + RE)
            e[:, g0 - lo : g1 - lo, :] = x[b].reshape(C, H, W)[:, g0:g1, :]
            return np.ascontiguousarray(e.reshape(C, NE).astype(bf16))

        rows = np.ones(RE, f32)
        if rg == 0:
            rows[0] = 0.0
        if rg == 3:
            rows[RE - 1] = 0.0
        mask = np.broadcast_to(np.repeat(rows, W)[None, :], (128, NE)).astype(bf16)

        in_maps.append(
            {
                "x1f": np.ascontiguousarray(x1b[b]),
                "x2f": np.ascontiguousarray(x2b[b]),
                "x1e": ext(x1),
                "x2e": ext(x2),
                "maskd": np.ascontiguousarray(mask),
                "wqkd": wqk,
                "wvd": wv2,
                "wcatd": wcat2,
                "bmiscd": bmisc,
            }
        )
    return in_maps


def _assemble(results):
    f32 = np.float32
    feat_sum = np.empty((B, C, H, W), f32)
    out1 = np.empty((B, C, H, W), f32)
    out2 = np.empty((B, C, H, W), f32)
    for core in range(N_CORES):
        b, rg = core // 4, core % 4
        r0 = 16 * rg
        r = results[core]
        out1[b, :, r0 : r0 + 16] = np.asarray(r["o1"], f32).reshape(C, 16, W)
        out2[b, :, r0 : r0 + 16] = np.asarray(r["o2"], f32).reshape(C, 16, W)
        feat_sum[b, :, r0 : r0 + 16] = np.asarray(r["feat"], f32).reshape(C, 16, W)
    return feat_sum, out1, out2


def _get_program(gamma: float):
    if gamma not in _PROG_CACHE:
        _PROG_CACHE[gamma] = _build_program(gamma)
    return _PROG_CACHE[gamma]


def kernel(input1, input2, Wq, bq, Wk, bk, Wv, bv, gamma, Wcat, bn_gamma, bn_beta):
    g = float(np.asarray(gamma).reshape(-1)[0])
    nc = _get_program(g)
    in_maps = _prep_inputs(
        input1, input2, Wq, bq, Wk, bk, Wv, bv, gamma, Wcat, bn_gamma, bn_beta
    )
    res = run_bass_kernel_spmd(nc, in_maps, core_ids=list(range(N_CORES)))
    return _assemble(res.results)


def run_traced(inputs):
    """For test.py: run, return (outputs, exec_time_ns)."""
    g = float(np.asarray(inputs["gamma"]).reshape(-1)[0])
    nc = _get_program(g)
    in_maps = _prep_inputs(**inputs)
    res = run_bass_kernel_spmd(nc, in_maps, core_ids=list(range(N_CORES)))
    return _assemble(res.results), res.exec_time_ns
